# revision 24
# baseline (speedup 1.0000x reference)
"""Trainium2 Bass kernel for nn_LocalInteractionsLayer.

Reference computation:
    seq_pairs [B=16, C=8, L=4096, 2] f32
    top = seq_pairs[..., 0]; bot = seq_pairs[..., 1]
    out[b, p, c*225 + i*15 + j] = top[b, c, p+i] * bot[b, c, p+j]
    for p in [0, P), i,j in [0, 15), P = L - 14 = 4082
    -> out [16, 4082, 1800] f32 (~470 MB; heavily output-write bound).

Strategy (v3, row-packed):
  - Data-parallel over batch: 2 batches per core on 8 cores.
  - All device I/O in float16 (2e-2 rel-err budget dwarfs f16's ~4e-4),
    halving the dominant output-store traffic vs f32.
  - Row packing: SBUF partition p of a position-tile holds TWO adjacent
    output rows (2p, 2p+1), so every store descriptor covers 7200
    contiguous bytes of DRAM. Measured on HW: 7200B descriptors sustain
    ~334 GB/s vs only ~228 GB/s for the 3600B descriptors a plain f16
    row-per-partition layout produces (and 14400B descs are slow again).
  - 16-wide host-prebuilt windows: rows 2p and 2p+1 share one 16-value
    window per (channel, operand), so the host-side expansion is 8x
    instead of 15x (2.1 MB of loads per core instead of 3.9 MB).
  - Compute per tile (256 positions, [128, 2*1800] f16):
      * row 2p's full 15x15 outer block: one DVE tensor_mul (1800/part)
      * row 2p+1 reuses row 2p's block shifted by (1,1): the shared 14x14
        interior is a scalar-engine (ACT) copy, only the new L-shape
        (i=14 row, j=14 column) is computed by two small DVE muls.
    => DVE ~83 us, ACT ~54 us, both hidden under the ~92 us DMA stream.
  - Stores ride the SP HWDGE ring (one 900 KB DMA per tile); loads ride
    the ACT ring as a 14-tile (7168B-desc) + 2-tile load per batch.
  Measured: ~96 us/iter steady-state (median slope), ~2.1x the f32
  baseline's 199 us; stores at the ~334 GB/s descriptor-limited wall.
"""

import sys

if "/opt/trn_rl_repo" not in sys.path:
    sys.path.insert(0, "/opt/trn_rl_repo")

import numpy as np
from numpy.lib.stride_tricks import sliding_window_view

import concourse.tile as tile
from concourse import bacc, mybir
from concourse.bass_utils import run_bass_kernel_spmd

W = 15            # window length (2*7+1)
WPAD = W - 1
U = 16            # shared window width for a row pair (W + 1)
B, C, L = 16, 8, 4096
P = L - WPAD      # 4082 valid output positions
BLK = W * W       # 225
FREE = C * BLK    # 1800
NCORES = 8
BPC = B // NCORES  # batches per core = 2
RP = 2             # output rows packed per SBUF partition
TPOS = RP * 128    # positions per tile = 256
NT = L // TPOS     # 16 position-tiles per batch (last one partial: 242 rows)
TW = 2 * C * U     # per-tile operand window elems per partition = 256
BW = NT * TW       # per-batch operand window elems per partition = 4096

_BUILD_CACHE: dict = {}


def _build(loop_iters: int = 1, in_bufs: int = 4, out_bufs: int = 6,
           first_fast: bool = True, rects_on_pool: bool = False,
           copies_on: str = "scalar", store_ring: str = "sync",
           load_split: tuple = (14,)):
    """Build + compile the per-core Bacc program (identical on all 8 cores)."""
    nc = bacc.Bacc("TRN2", target_bir_lowering=False, debug=False, num_devices=NCORES)
    dt = mybir.dt.float16

    # inw[b, p, tq*TW + s*C*U + c*U + u] = window value u for operand s,
    # channel c, output rows (2p, 2p+1) of tile tq. Flat per-batch layout so
    # load-group boundaries (hence DMA descriptor sizes) are free to choose:
    # the default (14,) split gives a 7168B-desc load + a 1024B-desc load,
    # near the measured ~7200B descriptor bandwidth sweet spot.
    inw_d = nc.dram_tensor("inw", [BPC, 128, BW], dt, kind="ExternalInput")
    bounds = [0, *[s * TW for s in load_split], BW]
    groups = [(bounds[i], bounds[i + 1]) for i in range(len(bounds) - 1)]
    out_d = nc.dram_tensor("out", [BPC, P, FREE], dt, kind="ExternalOutput")

    with tile.TileContext(nc) as tc:
        with (
            tc.tile_pool(name="inp", bufs=in_bufs) as inp,
            tc.tile_pool(name="outp", bufs=out_bufs) as outp,
        ):
            def compute_and_store(opw, b, t):
                """opw: [128, TW] operand view (s, c, u); tile t of batch b."""
                ot = outp.tile([128, RP * FREE], dt, tag="ot")
                v = opw.rearrange("p (s c u) -> p s c u", s=2, c=C)
                o4 = ot[:].rearrange("p (r c i j) -> p r c i j", r=RP, c=C, i=W)
                # Row 2p: full 15x15 outer block, one big DVE mul.
                a0 = v[:, 0, :, 0:W].unsqueeze(3).broadcast_to((128, C, W, W))
                b0 = v[:, 1, :, 0:W].unsqueeze(2).broadcast_to((128, C, W, W))
                nc.vector.tensor_mul(o4[:, 0], a0, b0)
                # Row 2p+1, new L-shape (window positions shifted by +1):
                reng = nc.gpsimd if rects_on_pool else nc.vector
                # rect A: i = 14 row -> top u=15, bot u=1..15
                aA = v[:, 0, :, W].unsqueeze(2).broadcast_to((128, C, W))
                bA = v[:, 1, :, 1:U]
                reng.tensor_mul(o4[:, 1, :, W - 1, :], aA, bA)
                # rect B: j = 14 col, i = 0..13 -> top u=1..14, bot u=15
                aB = v[:, 0, :, 1:W]
                bB = v[:, 1, :, W].unsqueeze(2).broadcast_to((128, C, W - 1))
                reng.tensor_mul(o4[:, 1, :, 0 : W - 1, W - 1], aB, bB)
                # Row 2p+1 shared 14x14 interior = row 2p block shifted (1,1).
                ceng = {"scalar": nc.scalar, "vector": nc.vector,
                        "gpsimd": nc.gpsimd}[copies_on]
                ceng.copy(
                    o4[:, 1, :, 0 : W - 1, 0 : W - 1],
                    o4[:, 0, :, 1:W, 1:W],
                )
                # Store: 2 adjacent DRAM rows per partition -> 7200B descs.
                base = t * TPOS
                npart = min(128, (P - base) // RP)
                dst = out_d[b, base : base + RP * npart, :].rearrange(
                    "(p r) f -> p (r f)", r=RP
                )
                if store_ring == "alt":
                    st = (nc.sync, nc.scalar)[t % 2]
                elif store_ring == "alt3":
                    st = (nc.sync, nc.scalar, nc.gpsimd)[t % 3]
                else:
                    st = nc.sync
                st.dma_start(dst, ot[:npart, :])

            def _body(_it=None):
                for b in range(BPC):
                    for gi, (e0, e1) in enumerate(groups):
                        starter = first_fast and b == 0 and gi == 0
                        if starter:
                            # Tiny dedicated load of tile 0's operands so the
                            # first store enters the DMA stream early.
                            inwt0 = inp.tile([128, TW], dt, tag="inwS")
                            nc.scalar.dma_start(inwt0[:], inw_d[0, :, 0:TW])
                            compute_and_store(inwt0[:], 0, 0)
                        inwt = inp.tile([128, e1 - e0], dt, tag=f"inw{gi}")
                        nc.scalar.dma_start(inwt[:], inw_d[b, :, e0:e1])
                        for tq in range(e0 // TW, e1 // TW):
                            if starter and tq == 0:
                                continue
                            compute_and_store(
                                inwt[:, tq * TW - e0 : (tq + 1) * TW - e0],
                                b, tq,
                            )

            if loop_iters == 1:
                _body()
            else:
                with tc.For_i(0, loop_iters, 1) as it:
                    _body(it)
    nc.compile()
    return nc


def _get_built(loop_iters: int = 1):
    nc = _BUILD_CACHE.get(loop_iters)
    if nc is None:
        nc = _build(loop_iters)
        _BUILD_CACHE[loop_iters] = nc
    return nc


def _prep(seq_pairs: np.ndarray) -> np.ndarray:
    """Host-side 16-wide window expansion into the device layout (f16).

    inw[b, p, ((tq*2 + s)*C + c)*U + u] = seq_pairs[b, c, tq*256 + 2p + u, s]
    (positions past L-1 read zero padding; rows past P-1 are never stored).
    """
    sp = np.ascontiguousarray(seq_pairs, dtype=np.float32)
    padded = np.zeros((B, C, L + WPAD, 2), np.float32)
    padded[:, :, :L] = sp
    win16 = sliding_window_view(padded, U, axis=2)  # [B, C, 4095, 2, U]
    ev = win16[:, :, 0 : NT * TPOS : RP]            # [B, C, 2048, 2, U]
    v = ev.reshape(B, C, NT, 128, 2, U)
    v = v.transpose(0, 3, 2, 4, 1, 5)               # [b, p, tq, s, c, u]
    return np.ascontiguousarray(v, dtype=np.float16).reshape(B, 128, BW)


def kernel(seq_pairs: np.ndarray) -> np.ndarray:
    assert tuple(np.shape(seq_pairs)) == (B, C, L, 2), (
        f"expected seq_pairs shape {(B, C, L, 2)}, got {np.shape(seq_pairs)}"
    )
    inw = _prep(seq_pairs)
    nc = _get_built()
    in_maps = [{"inw": inw[k * BPC : (k + 1) * BPC]} for k in range(NCORES)]
    last_err = None
    for _attempt in range(3):
        try:
            res = run_bass_kernel_spmd(nc, in_maps, list(range(NCORES))).results
            break
        except Exception as err:  # transient axon/PJRT hiccups — retry
            last_err = err
    else:
        raise last_err
    out = np.concatenate([res[k]["out"] for k in range(NCORES)], axis=0)
    return np.ascontiguousarray(out.astype(np.float32))
